# revision 32
# baseline (speedup 1.0000x reference)
"""Canny edge detector kernel for Trainium2, 8-core data parallel.

Full inputs: imgs [16, 3, 1024, 1024] f32. Output: same shape, f32 {0,1}.

Per pixel: 4 directional central differences (E-W, NW-SE, N-S, NE-SW),
edge = 1 if any |diff| >= 15/255.

Sharding: 48 independent [1024,1024] images -> 6 per core.

Per-core plan (per 126-row block):
  X tile [128, 1026]: rows r0-1..r0+126 (partition p = row r0-1+p), zero halo cols.
  B tile [126, 1026] = X[2:128] via SBUF->SBUF partition-shifted DMA.
  At partition q (out row i = r0+q): x[i-1] = X[q], x[i+1] = B[q].
    t45  = X(j-1) - B(j+1)      (DVE)
    t90  = X(j)   - B(j)        (DVE)
    t135 = X(j+1) - B(j-1)      (GPSIMD)
    t0   = X(j-1) - X(j+1) on partitions 0..126 (center-row aligned, +1 off) (GPSIMD)
  a_i = |t_i| (ACT Abs), b_i = (a_i >= T) as bf16 (DVE tensor_scalar)
  s = I@b45 + I@b90 + I@b135 + Wm1@b0 in PSUM (PE bf16 matmuls; Wm1 realigns b0)
  out = (s >= 0.5) f32 (DVE is_ge / ACT Sign+Relu), DMA to DRAM.
Rows 1008..1023 of all 6 images are packed into one extra block (partition
groups of 18 per image).
"""

import numpy as np
import ml_dtypes
from contextlib import ExitStack

import concourse.bass as bass
import concourse.tile as tile
from concourse import bacc, mybir
from concourse.bass_utils import run_bass_kernel_spmd
from concourse._compat import axon_active

f32 = mybir.dt.float32
bf16 = mybir.dt.bfloat16

N_CORES = 8
H = W = 1024
IMGS_PER_CORE = 6
RPC = IMGS_PER_CORE * H  # rows per core
THRESH = float(np.float32(15.0 / 255.0))
BLK = 126  # output rows per full block
NFULL = 8  # full blocks per image (8*126 = 1008)
TAIL = H - NFULL * BLK  # 16
GRP = 18  # partition group per image in the packed tail block

_CACHE = {}


def _emit_block(nc, tc, ctx, pools, imgs_in, imgs_out, wi, wm1, bm05, zcol2, im, b):
    xpool, bpool, tpool, apool, ipool, opool, pspool = pools
    AF = mybir.ActivationFunctionType
    OP = mybir.AluOpType

    X = xpool.tile([128, 1026], f32, name=f"x_{im}_{b}", tag="X")
    packed = b == NFULL
    if packed:
        # zero everything: halo cols, pad rows, unused partitions
        nc.gpsimd.memset(X[:], 0.0)
        for j in range(IMGS_PER_CORE):
            r = j * H + (H - TAIL - 1)  # 1007
            nc.sync.dma_start(
                X[j * GRP : j * GRP + TAIL + 1, 1:1025], imgs_in[r : r + TAIL + 1, :]
            )
    else:
        # zero halo columns 0 and 1025
        nc.sync.dma_start(X[:, 0:1], zcol2[:, 0:1])
        nc.sync.dma_start(X[:, 1025:1026], zcol2[:, 1:2])
        r0 = im * H + b * BLK
        if b == 0:
            nc.gpsimd.memset(X[0:1, :], 0.0)
            nc.sync.dma_start(X[1:128, 1:1025], imgs_in[r0 : r0 + 127, :])
        else:
            nc.sync.dma_start(X[:, 1:1025], imgs_in[r0 - 1 : r0 + 127, :])

    B = bpool.tile([126, 1026], f32, name=f"b_{im}_{b}", tag="B")
    nc.sync.dma_start(B[:], X[2:128, :])

    t0 = tpool.tile([127, 1024], f32, name=f"t0_{im}_{b}", tag="t0")
    nc.gpsimd.tensor_tensor(t0[:], X[0:127, 0:1024], X[0:127, 2:1026], OP.subtract)
    t135 = tpool.tile([126, 1024], f32, name=f"t135_{im}_{b}", tag="t135")
    nc.gpsimd.tensor_tensor(t135[:], X[0:126, 2:1026], B[:, 0:1024], OP.subtract)
    t45 = tpool.tile([126, 1024], f32, name=f"t45_{im}_{b}", tag="t45")
    nc.vector.tensor_tensor(t45[:], X[0:126, 0:1024], B[:, 2:1026], OP.subtract)
    t90 = tpool.tile([126, 1024], f32, name=f"t90_{im}_{b}", tag="t90")
    nc.vector.tensor_tensor(t90[:], X[0:126, 1:1025], B[:, 1:1025], OP.subtract)

    a0 = apool.tile([127, 1024], f32, name=f"a0_{im}_{b}", tag="a0")
    nc.scalar.activation(a0[:], t0[:], AF.Abs)
    a135 = apool.tile([126, 1024], f32, name=f"a135_{im}_{b}", tag="a135")
    nc.scalar.activation(a135[:], t135[:], AF.Abs)
    a45 = apool.tile([126, 1024], f32, name=f"a45_{im}_{b}", tag="a45")
    nc.scalar.activation(a45[:], t45[:], AF.Abs)
    a90 = apool.tile([126, 1024], f32, name=f"a90_{im}_{b}", tag="a90")
    nc.scalar.activation(a90[:], t90[:], AF.Abs)

    b0 = ipool.tile([127, 1024], bf16, name=f"i0_{im}_{b}", tag="i0")
    nc.vector.tensor_scalar(b0[:], a0[:], THRESH, None, OP.is_ge)
    b135 = ipool.tile([126, 1024], bf16, name=f"i135_{im}_{b}", tag="i135")
    nc.vector.tensor_scalar(b135[:], a135[:], THRESH, None, OP.is_ge)
    b45 = ipool.tile([126, 1024], bf16, name=f"i45_{im}_{b}", tag="i45")
    nc.vector.tensor_scalar(b45[:], a45[:], THRESH, None, OP.is_ge)
    b90 = ipool.tile([126, 1024], bf16, name=f"i90_{im}_{b}", tag="i90")
    nc.vector.tensor_scalar(b90[:], a90[:], THRESH, None, OP.is_ge)

    o = opool.tile([126, 1024], f32, name=f"o_{im}_{b}", tag="o")
    for h in range(2):
        cs = slice(512 * h, 512 * h + 512)
        c0 = 512 * h
        s_ps = pspool.tile([126, 512], f32, name=f"s_{im}_{b}_{h}", tag=f"s{h}")
        nc.tensor.matmul(s_ps[:], wi[0:126, 0:126], b45[:, cs], start=True, stop=False)
        nc.tensor.matmul(s_ps[:], wi[0:126, 0:126], b90[:, cs], start=False, stop=False)
        nc.tensor.matmul(s_ps[:], wi[0:126, 0:126], b135[:, cs], start=False, stop=False)
        nc.tensor.matmul(s_ps[:], wm1[0:127, 0:126], b0[:, cs], start=False, stop=True)
        if h == 0:
            nc.vector.tensor_scalar(o[:, cs], s_ps[:], 0.5, None, OP.is_ge)
        else:
            # s >= 0 always (sum of 0/1 indicators), so Sign(s) is exactly
            # the 0/1 edge answer in one ACT op.
            nc.scalar.activation(o[:, cs], s_ps[:], AF.Sign)

    if packed:
        for j in range(IMGS_PER_CORE):
            nc.sync.dma_start(
                imgs_out[j * H + H - TAIL : (j + 1) * H, :],
                o[j * GRP : j * GRP + TAIL, :],
            )
    else:
        r0 = im * H + b * BLK
        nc.sync.dma_start(imgs_out[r0 : r0 + BLK, :], o[:])


def _build():
    if "nc" in _CACHE:
        return _CACHE["nc"]
    nc = bacc.Bacc("TRN2", target_bir_lowering=False, debug=not axon_active())
    imgs_in = nc.declare_dram_parameter("imgs", [RPC, W], f32, isOutput=False)
    zcol2 = nc.declare_dram_parameter("zcol2", [128, 2], f32, isOutput=False)
    wts = nc.declare_dram_parameter("wts", [2, 128, 128], bf16, isOutput=False)
    imgs_out = nc.declare_dram_parameter("out", [RPC, W], f32, isOutput=True)

    with tile.TileContext(nc) as tc, ExitStack() as ctx:
        wpool = ctx.enter_context(tc.tile_pool(name="wp", bufs=1))
        xpool = ctx.enter_context(tc.tile_pool(name="xp", bufs=4))
        bpool = ctx.enter_context(tc.tile_pool(name="bp", bufs=4))
        tpool = ctx.enter_context(tc.tile_pool(name="tp", bufs=2))
        apool = ctx.enter_context(tc.tile_pool(name="ap", bufs=2))
        ipool = ctx.enter_context(tc.tile_pool(name="ip", bufs=2))
        opool = ctx.enter_context(tc.tile_pool(name="op", bufs=4))
        pspool = ctx.enter_context(tc.tile_pool(name="ps", bufs=2, space="PSUM"))
        pools = (xpool, bpool, tpool, apool, ipool, opool, pspool)

        wi = wpool.tile([128, 128], bf16, name="wi")
        nc.sync.dma_start(wi[:], wts[0])
        wm1 = wpool.tile([128, 128], bf16, name="wm1")
        nc.sync.dma_start(wm1[:], wts[1])
        bm05 = wpool.tile([128, 1], f32, name="bm05")
        nc.gpsimd.memset(bm05[:], -0.5)

        for im in range(IMGS_PER_CORE):
            for b in range(NFULL):
                _emit_block(
                    nc, tc, ctx, pools, imgs_in, imgs_out, wi, wm1, bm05, zcol2, im, b
                )
        _emit_block(
            nc, tc, ctx, pools, imgs_in, imgs_out, wi, wm1, bm05, zcol2, 0, NFULL
        )

    nc.compile()
    _CACHE["nc"] = nc
    return nc


def _weights():
    wi = np.eye(128, dtype=np.float32)
    wm1 = np.zeros((128, 128), np.float32)
    for m in range(127):
        wm1[m + 1, m] = 1.0  # out[m] = rhs[m+1]
    return np.stack([wi, wm1]).astype(ml_dtypes.bfloat16)


def _run(imgs: np.ndarray, trace: bool = False):
    imgs = np.asarray(imgs)
    dt_in = imgs.dtype
    Bt, C = imgs.shape[0], imgs.shape[1]
    flat = np.ascontiguousarray(
        imgs.astype(np.float32).reshape(Bt * C, H, W)
    )  # [48, H, W]
    per = Bt * C // N_CORES
    nc = _build()
    wts = _weights()
    zc = np.zeros((128, 2), np.float32)
    in_maps = [
        {
            "imgs": np.ascontiguousarray(
                flat[c * per : (c + 1) * per].reshape(per * H, W)
            ),
            "zcol2": zc,
            "wts": wts,
        }
        for c in range(N_CORES)
    ]
    res = run_bass_kernel_spmd(nc, in_maps, list(range(N_CORES)), trace=trace)
    out = np.stack([res.results[c]["out"].reshape(per, H, W) for c in range(N_CORES)])
    return out.reshape(Bt, C, H, W).astype(dt_in), res


def kernel(imgs: np.ndarray) -> np.ndarray:
    return _run(imgs)[0]
